# revision 5
# baseline (speedup 1.0000x reference)
"""Trainium2 Bass kernel for ContinuallyLearningPrototypes predict (vq_codebook).

Computes, for X (N,D), unit-norm prototypes (P,D), per-prototype threshold
sim_th (constant 0.45), prototype class labels (P,) in [0,C):

    scores[n,c] = max over prototypes p with label c of
                  thresholded( <prototypes[p], X[n]/||X[n]||> )
    where thresholded(s) = s if s >= th else 0, and empty classes give 0.

Strategy (8 NeuronCores, data-parallel over N):
  - Shard X rows across 8 cores (4096 rows each); replicate prototypes.
  - Host marshaling: sort prototypes by class, pad each class to a multiple
    of 16 columns with zero prototypes, transpose to (D, P'). Zero columns
    yield raw sims 0 which sit below any positive threshold, so they never
    affect results.
  - Device per 128-sample tile:
      PE:  fp32 matmul  sims_raw = X_tile @ P'^T   (exact fp32 - the
           threshold at 0.45 makes reduced-precision matmuls produce
           threshold flips worth ~0.45 absolute error each).
      ACT: evict PSUM -> SBUF bf16 with fused r = relu(sims_raw - 0.45*||x||)
           (per-partition bias).  relu/-t shift and normalization are
           monotone, so class-max commutes with them; after the shift bf16
           rounding cannot flip a threshold decision (relu(x)>0 iff
           bf16(relu(x))>0) and its error is relative to the *shifted*
           value, which is what makes the bf16 fold tree numerically safe.
      DVE: per-class max via tensor_tensor max fold tree (2x bf16 mode)
           + final 3D-AP reduce_max, then scores = m*inv_norm + 0.45*(m>0).
  - Row norms computed on device from X (batched; Newton-refined sqrt and
    hardware iterative-divide reciprocal for fp32-accurate thresholds).

The Bass program is specialized at call time to the label multiset (class
column layout baked in as immediates); all arithmetic on tensors happens
on-device.
"""

import os
import sys
from contextlib import ExitStack

import numpy as np

sys.path.insert(0, "/opt/trn_rl_repo")

import concourse.bass as bass
import concourse.bacc as bacc
import concourse.tile as tile
from concourse import mybir
from concourse.bass_utils import run_bass_kernel_spmd

f32 = mybir.dt.float32
bf16 = mybir.dt.bfloat16
AF = mybir.ActivationFunctionType
OP = mybir.AluOpType
AX = mybir.AxisListType

N_CORES = 8
N_FULL, D, P_TOT, C = 32768, 64, 4096, 100
N_SH = N_FULL // N_CORES          # samples per core
T_TILES = N_SH // 128             # 128-sample tiles per core
GEN_CAP = 2048                    # PSUM generation width (4 banks of fp32)
PAD = 16                          # class column padding granularity
MM_N = 512                        # fp32 matmul moving-dim limit
FOLD_LEVELS = 2                   # TT-max fold levels before reduce_max

# filled by the most recent kernel() call when tracing is enabled
LAST_EXEC_NS = None
LAST_MEAN_EXEC_NS = None


def _plan_layout(proto_labels):
    """Sort classes by padded width, pack into PSUM generations.

    Returns (gens, order, cls_slots, Pp):
      gens: list of (width, buckets) where buckets is a list of
            (S, n_classes, col_off_in_gen, slot0)
      order: prototype permutation (sorted layout column -> original index),
             with -1 for zero-pad columns
      cls_slots: list of class ids in slot order (slot i = output column i)
      Pp: total padded column count
    """
    labels = np.asarray(proto_labels).astype(np.int64)
    valid = labels >= 0
    counts = np.bincount(labels[valid], minlength=C)[:C]
    present = np.nonzero(counts)[0]
    padded = ((counts + PAD - 1) // PAD) * PAD
    cls_order = sorted(present, key=lambda c: (padded[c], c))

    # prototype indices per class
    idx_of = {c: np.nonzero(labels == c)[0] for c in present}

    gens = []
    order_cols = []
    cur_w = 0
    cur_buckets = []  # accumulated (S, class_list) runs for current gen
    slot = 0

    def close_gen():
        nonlocal cur_w, cur_buckets, slot
        if cur_w == 0:
            return
        buckets = []
        off = 0
        for S, cls_list in cur_buckets:
            buckets.append((S, len(cls_list), off, slot))
            off += S * len(cls_list)
            slot += len(cls_list)
        gens.append((cur_w, buckets))
        cur_w = 0
        cur_buckets = []

    for c in cls_order:
        S = int(padded[c])
        if cur_w + S > GEN_CAP:
            close_gen()
        if cur_buckets and cur_buckets[-1][0] == S:
            cur_buckets[-1][1].append(c)
        else:
            cur_buckets.append([S, [c]])
        cur_w += S
        cols = np.full(S, -1, np.int64)
        cols[: counts[c]] = idx_of[c]
        order_cols.append(cols)
    close_gen()

    order = np.concatenate(order_cols) if order_cols else np.zeros(0, np.int64)
    Pp = int(order.shape[0])
    cls_slots = []
    for _, buckets in gens:
        pass
    # slots were assigned in cls_order sequence
    cls_slots = list(cls_order)
    return gens, order, cls_slots, Pp


def _fold_schedule(S):
    """Return (fold_widths, final_width): halve while even, >6, max levels."""
    widths = []
    cur = S
    while len(widths) < FOLD_LEVELS and cur % 2 == 0 and cur > 6:
        cur //= 2
        widths.append(cur)
    return widths, cur


def _build_nc(gens, Pp, nslot, th):
    nc = bacc.Bacc("TRN2", target_bir_lowering=False)
    xT_d = nc.declare_dram_parameter("xT", [D, N_SH], f32, isOutput=False)
    xN_d = nc.declare_dram_parameter("xN", [N_SH, D], f32, isOutput=False)
    pT_d = nc.declare_dram_parameter("pT", [D, Pp], f32, isOutput=False)
    out_d = nc.declare_dram_parameter("out", [N_SH, nslot], f32, isOutput=True)

    with tile.TileContext(nc) as tc, ExitStack() as ctx:
        cpool = ctx.enter_context(tc.tile_pool(name="const", bufs=1))
        npool = ctx.enter_context(tc.tile_pool(name="norm", bufs=1))
        rpool = ctx.enter_context(tc.tile_pool(name="r", bufs=3))
        fpool = ctx.enter_context(tc.tile_pool(name="fold", bufs=2))
        mpool = ctx.enter_context(tc.tile_pool(name="m", bufs=2))
        opool = ctx.enter_context(tc.tile_pool(name="o", bufs=2))
        pspool = ctx.enter_context(tc.tile_pool(name="ps", bufs=2, space="PSUM"))

        pT = cpool.tile([D, Pp], f32)
        nc.sync.dma_start(pT[:], pT_d[:])
        xT = cpool.tile([D, N_SH], f32)
        nc.sync.dma_start(xT[:], xT_d[:])
        xn = cpool.tile([128, T_TILES * D], f32)
        nc.sync.dma_start(
            xn[:].rearrange("p (t d) -> p t d", d=D),
            xN_d[:].rearrange("(t p) d -> p t d", p=128),
        )

        # --- batched row norms: ||x|| per sample, Newton-refined ---
        xsq = npool.tile([128, T_TILES * D], f32)
        nc.vector.tensor_mul(xsq[:], xn[:], xn[:])
        ss = npool.tile([128, T_TILES], f32)
        nc.vector.reduce_sum(
            ss[:], xsq[:].rearrange("p (t d) -> p t d", d=D), axis=AX.X
        )
        y0 = npool.tile([128, T_TILES], f32)
        nc.scalar.activation(y0[:], ss[:], AF.Sqrt)
        r0 = npool.tile([128, T_TILES], f32)
        nc.vector.reciprocal(r0[:], y0[:])
        a0 = npool.tile([128, T_TILES], f32)
        nc.vector.tensor_mul(a0[:], ss[:], r0[:])
        n1 = npool.tile([128, T_TILES], f32)
        nc.vector.tensor_add(n1[:], y0[:], a0[:])
        nrm = npool.tile([128, T_TILES], f32)
        nc.vector.tensor_scalar_mul(nrm[:], n1[:], 0.5)
        inv = npool.tile([128, T_TILES], f32)
        nc.vector.reciprocal(inv[:], nrm[:])
        tneg = npool.tile([128, T_TILES], f32)
        nc.vector.tensor_scalar_mul(tneg[:], nrm[:], -th)

        # --- main loop over 128-sample tiles ---
        for t in range(T_TILES):
            lhs = xT[:, t * 128 : (t + 1) * 128]
            m = mpool.tile([128, nslot], bf16, tag="m")
            col = 0
            for gw, buckets in gens:
                ps = pspool.tile([128, GEN_CAP], f32, tag="ps")
                for j0 in range(0, gw, MM_N):
                    w = min(MM_N, gw - j0)
                    nc.tensor.matmul(
                        ps[:, j0 : j0 + w],
                        lhs,
                        pT[:, col + j0 : col + j0 + w],
                        start=True,
                        stop=True,
                    )
                r = rpool.tile([128, GEN_CAP], bf16, tag="r")
                nc.scalar.activation(
                    r[:, :gw], ps[:, :gw], AF.Relu,
                    bias=tneg[:, t : t + 1], scale=1.0,
                )
                # fold scratch for this gen (shared by its buckets)
                scr = [
                    fpool.tile(
                        [128, GEN_CAP // (2 ** (l + 1))], bf16,
                        tag=f"f{l}", name=f"fold{l}",
                    )
                    for l in range(FOLD_LEVELS)
                ]
                scr_off = [0] * FOLD_LEVELS
                for S, nb, boff, slot0 in buckets:
                    widths, Sf = _fold_schedule(S)
                    src_ap = r[:, boff : boff + nb * S].rearrange(
                        "p (c s) -> p c s", s=S
                    )
                    cur_w = S
                    for li, wdt in enumerate(widths):
                        o = scr_off[li]
                        dst = scr[li][:, o : o + nb * wdt].rearrange(
                            "p (c s) -> p c s", s=wdt
                        )
                        nc.vector.tensor_max(
                            dst, src_ap[:, :, 0:wdt], src_ap[:, :, wdt:cur_w]
                        )
                        scr_off[li] = o + nb * wdt
                        src_ap = dst
                        cur_w = wdt
                    nc.vector.reduce_max(
                        m[:, slot0 : slot0 + nb], src_ap, axis=AX.X
                    )
                col += gw
            # scores = m * inv_norm + th * (m > 0)
            g01 = opool.tile([128, nslot], f32, tag="g01")
            nc.vector.tensor_single_scalar(g01[:], m[:], 0.0, op=OP.is_gt)
            ax = opool.tile([128, nslot], f32, tag="ax")
            nc.vector.tensor_scalar_mul(ax[:], m[:], inv[:, t : t + 1])
            sc = opool.tile([128, nslot], f32, tag="sc")
            nc.vector.scalar_tensor_tensor(
                sc[:], g01[:], th, ax[:], op0=OP.mult, op1=OP.add
            )
            nc.sync.dma_start(out_d[t * 128 : (t + 1) * 128, :], sc[:])

    nc.compile()
    return nc


def _marshal(X, prototypes, order):
    """Build per-core inputs + shared padded transposed prototypes."""
    Pp = order.shape[0]
    pTp = np.zeros((D, Pp), np.float32)
    real = order >= 0
    pTp[:, real] = np.ascontiguousarray(prototypes[order[real]].T)
    in_maps = []
    for c in range(N_CORES):
        Xs = np.ascontiguousarray(X[c * N_SH : (c + 1) * N_SH])
        in_maps.append(
            {
                "xT": np.ascontiguousarray(Xs.T),
                "xN": Xs,
                "pT": pTp,
            }
        )
    return in_maps


def kernel(X, prototypes, sim_th, proto_labels):
    global LAST_EXEC_NS, LAST_MEAN_EXEC_NS
    X = np.asarray(X, np.float32)
    prototypes = np.asarray(prototypes, np.float32)
    sim_th = np.asarray(sim_th, np.float32)
    labels = np.asarray(proto_labels)

    assert X.shape == (N_FULL, D) and prototypes.shape == (P_TOT, D)
    th_vals = np.unique(sim_th)
    assert th_vals.size == 1, "kernel assumes a single global threshold"
    th = float(th_vals[0])

    gens, order, cls_slots, Pp = _plan_layout(labels)
    nslot = len(cls_slots)
    nc = _build_nc(gens, Pp, nslot, th)
    in_maps = _marshal(X, prototypes, order)

    res = run_bass_kernel_spmd(nc, in_maps, list(range(N_CORES)))
    LAST_EXEC_NS = res.exec_time_ns
    LAST_MEAN_EXEC_NS = res.mean_exec_time_ns

    out_sorted = np.concatenate([res.results[i]["out"] for i in range(N_CORES)], 0)
    full = np.zeros((N_FULL, C), np.float32)
    full[:, np.asarray(cls_slots, np.int64)] = out_sorted
    return full


# revision 21
# speedup vs baseline: 28.4647x; 28.4647x over previous
"""Trainium2 Bass kernel for ContinuallyLearningPrototypes predict (vq_codebook).

Computes, for X (N,D), unit-norm prototypes (P,D), per-prototype threshold
sim_th (constant 0.45), prototype class labels (P,) in [0,C):

    scores[n,c] = max over prototypes p with label c of
                  thresholded( <prototypes[p], X[n]/||X[n]||> )
    where thresholded(s) = s if s >= th else 0, and empty classes give 0.

Strategy (8 NeuronCores, data-parallel over N):
  - Shard X rows across 8 cores (4096 rows each); replicate prototypes.
  - Host marshaling: sort prototypes by class, pad each class to a multiple
    of 16 columns with zero prototypes, transpose to (D, P'). Zero columns
    yield raw sims 0 which sit below any positive threshold, so they never
    affect results.
  - Device per 128-sample tile:
      PE:  fp32 matmul  sims_raw = X_tile @ P'^T   (exact fp32 - the
           threshold at 0.45 makes reduced-precision matmuls produce
           threshold flips worth ~0.45 absolute error each).
      ACT: evict PSUM -> SBUF bf16 with fused r = relu(sims_raw - 0.45*||x||)
           (per-partition bias).  relu/-t shift and normalization are
           monotone, so class-max commutes with them; after the shift bf16
           rounding cannot flip a threshold decision (relu(x)>0 iff
           bf16(relu(x))>0) and its error is relative to the *shifted*
           value, which is what makes the bf16 fold tree numerically safe.
      DVE: per-class max via tensor_tensor max fold tree (2x bf16 mode)
           + final 3D-AP reduce_max, then scores = m*inv_norm + 0.45*(m>0).
  - Row norms computed on device from X (batched; Newton-refined sqrt and
    hardware iterative-divide reciprocal for fp32-accurate thresholds).

The Bass program is specialized at call time to the label multiset (class
column layout baked in as immediates); all arithmetic on tensors happens
on-device.
"""

import sys
from contextlib import ExitStack

import numpy as np

sys.path.insert(0, "/opt/trn_rl_repo")

import concourse.bass as bass
import concourse.bacc as bacc
import concourse.tile as tile
from concourse import mybir
from concourse.bass_utils import run_bass_kernel_spmd

f32 = mybir.dt.float32
bf16 = mybir.dt.bfloat16
AF = mybir.ActivationFunctionType
OP = mybir.AluOpType
AX = mybir.AxisListType

N_CORES = 8
N_FULL, D, P_TOT, C = 32768, 64, 4096, 100
N_SH = N_FULL // N_CORES          # samples per core
T_TILES = N_SH // 128             # 128-sample tiles per core
GEN_CAP = 1024                    # PSUM generation width (2 banks of fp32; 2 tags x 2 bufs = 8 banks)
PAD = 8                           # class column padding granularity
MM_N = 512                        # fp32 matmul moving-dim limit
FOLD_LEVELS = 2                   # TT-max fold levels before reduce_max
MM_DTYPE = "fp32"                 # "fp32" (4 cyc/row) or "fp16x3" hi/lo split (3 cyc/row)
DUP_PE = 1                        # probe: emit each matmul this many times
DUP_ACT = 1                       # probe: emit each evict this many times
DUP_DVE = 1                       # probe: emit fold/reduce ops this many times
DVE_GENS = ()                     # gen indices reduced directly on DVE from PSUM
BIG_R = True                      # evict into one per-tile r buffer; global-bucket folds

# filled by the most recent kernel() call when tracing is enabled
LAST_EXEC_NS = None
LAST_MEAN_EXEC_NS = None


def _plan_layout(proto_labels, gen_cap=GEN_CAP):
    """Sort classes by padded width, pack into PSUM generations.

    Returns (gens, order, cls_slots, Pp):
      gens: list of (width, buckets) where buckets is a list of
            (S, n_classes, col_off_in_gen, slot0)
      order: prototype permutation (sorted layout column -> original index),
             with -1 for zero-pad columns
      cls_slots: list of class ids in slot order (slot i = output column i)
      Pp: total padded column count
    """
    labels = np.asarray(proto_labels).astype(np.int64)
    valid = labels >= 0
    counts = np.bincount(labels[valid], minlength=C)[:C]
    present = np.nonzero(counts)[0]
    padded = ((counts + PAD - 1) // PAD) * PAD
    cls_order = sorted(present, key=lambda c: (padded[c], c))

    # prototype indices per class
    idx_of = {c: np.nonzero(labels == c)[0] for c in present}

    gens = []
    order_cols = []
    cur_w = 0
    cur_buckets = []  # accumulated (S, class_list) runs for current gen
    slot = 0

    def close_gen():
        nonlocal cur_w, cur_buckets, slot
        if cur_w == 0:
            return
        buckets = []
        off = 0
        for S, cls_list in cur_buckets:
            buckets.append((S, len(cls_list), off, slot))
            off += S * len(cls_list)
            slot += len(cls_list)
        gens.append((cur_w, buckets))
        cur_w = 0
        cur_buckets = []

    for c in cls_order:
        S = int(padded[c])
        if cur_w + S > gen_cap:
            close_gen()
        if cur_buckets and cur_buckets[-1][0] == S:
            cur_buckets[-1][1].append(c)
        else:
            cur_buckets.append([S, [c]])
        cur_w += S
        cols = np.full(S, -1, np.int64)
        cols[: counts[c]] = idx_of[c]
        order_cols.append(cols)
    close_gen()

    order = np.concatenate(order_cols) if order_cols else np.zeros(0, np.int64)
    Pp = int(order.shape[0])
    # slots were assigned in cls_order sequence
    cls_slots = list(cls_order)
    # global buckets: runs of equal padded width across the whole layout
    gbuckets = []
    goff = 0
    gslot = 0
    for c in cls_order:
        S = int(padded[c])
        if gbuckets and gbuckets[-1][0] == S:
            S0, nb, o0, s0 = gbuckets[-1]
            gbuckets[-1] = (S0, nb + 1, o0, s0)
        else:
            gbuckets.append((S, 1, goff, gslot))
        goff += S
        gslot += 1
    return gens, order, cls_slots, Pp, gbuckets


def _fold_schedule(S):
    """Return (fold_widths, final_width): halve while even, >6, max levels."""
    widths = []
    cur = S
    while len(widths) < FOLD_LEVELS and cur % 2 == 0 and cur > 6:
        cur //= 2
        widths.append(cur)
    return widths, cur


def _build_nc(gens, Pp, nslot, th, reps=1, mode="full", gbuckets=()):
    nc = bacc.Bacc("TRN2", target_bir_lowering=False)
    fp16 = mybir.dt.float16
    xN_d = nc.declare_dram_parameter("xN", [N_SH, D], f32, isOutput=False)
    out_d = nc.declare_dram_parameter("out", [N_SH, nslot], f32, isOutput=True)
    if MM_DTYPE == "fp32":
        xT_d = nc.declare_dram_parameter("xT", [2 * D, N_SH], f32, isOutput=False)
        pT_d = nc.declare_dram_parameter("pT", [2 * D, Pp], f32, isOutput=False)
    else:
        xTh_d = nc.declare_dram_parameter("xTh", [2 * D, N_SH], fp16, isOutput=False)
        xTl_d = nc.declare_dram_parameter("xTl", [2 * D, N_SH], fp16, isOutput=False)
        pTh_d = nc.declare_dram_parameter("pTh", [2 * D, Pp], fp16, isOutput=False)
        pTl_d = nc.declare_dram_parameter("pTl", [2 * D, Pp], fp16, isOutput=False)

    with tile.TileContext(nc) as tc, ExitStack() as ctx:
        rep_ctx = tc.For_i(0, reps, 1) if reps > 1 else None
        if rep_ctx is not None:
            rep_ctx.__enter__()
        cpool = ctx.enter_context(tc.tile_pool(name="const", bufs=1))
        npool = ctx.enter_context(tc.tile_pool(name="norm", bufs=1))
        rpool = ctx.enter_context(tc.tile_pool(name="r", bufs=3))
        fpool = ctx.enter_context(tc.tile_pool(name="fold", bufs=2))
        mpool = ctx.enter_context(tc.tile_pool(name="m", bufs=2))
        opool = ctx.enter_context(tc.tile_pool(name="o", bufs=2))
        pspool = ctx.enter_context(tc.tile_pool(name="ps", bufs=2, space="PSUM"))

        if MM_DTYPE == "fp32":
            pT = cpool.tile([2 * D, Pp], f32)
            nc.sync.dma_start(pT[:], pT_d[:])
            xT = cpool.tile([2 * D, N_SH], f32)
            nc.sync.dma_start(xT[:], xT_d[:])
        else:
            pTh = cpool.tile([2 * D, Pp], fp16)
            nc.sync.dma_start(pTh[:], pTh_d[:])
            pTl = cpool.tile([2 * D, Pp], fp16)
            nc.sync.dma_start(pTl[:], pTl_d[:])
            xTh = cpool.tile([2 * D, N_SH], fp16)
            nc.sync.dma_start(xTh[:], xTh_d[:])
            xTl = cpool.tile([2 * D, N_SH], fp16)
            nc.sync.dma_start(xTl[:], xTl_d[:])
        xn = cpool.tile([128, T_TILES * D], f32)
        nc.sync.dma_start(
            xn[:].rearrange("p (t d) -> p t d", d=D),
            xN_d[:].rearrange("(t p) d -> p t d", p=128),
        )

        # --- batched row norms: ||x|| per sample, Newton-refined ---
        xsq = npool.tile([128, T_TILES * D], f32)
        nc.vector.tensor_mul(xsq[:], xn[:], xn[:])
        ss = npool.tile([128, T_TILES], f32)
        nc.vector.reduce_sum(
            ss[:], xsq[:].rearrange("p (t d) -> p t d", d=D), axis=AX.X
        )
        y0 = npool.tile([128, T_TILES], f32)
        nc.scalar.activation(y0[:], ss[:], AF.Sqrt)
        r0 = npool.tile([128, T_TILES], f32)
        nc.vector.reciprocal(r0[:], y0[:])
        a0 = npool.tile([128, T_TILES], f32)
        nc.vector.tensor_mul(a0[:], ss[:], r0[:])
        n1 = npool.tile([128, T_TILES], f32)
        nc.vector.tensor_add(n1[:], y0[:], a0[:])
        nrm = npool.tile([128, T_TILES], f32)
        nc.vector.tensor_scalar_mul(nrm[:], n1[:], 0.5)
        inv = npool.tile([128, T_TILES], f32)
        nc.vector.reciprocal(inv[:], nrm[:])
        tneg = npool.tile([128, T_TILES], f32)
        nc.vector.tensor_scalar_mul(tneg[:], nrm[:], -th)

        def post_matmul_dve(t, half, ps, gw, buckets, m):
            """Direct path: per-class reduce_max straight from PSUM (fp32),
            then fused (u - 0.45*||x||) relu into bf16 m slots on DVE.
            Skips the ACT eviction entirely for this generation."""
            for S, nb, boff, slot0 in buckets:
                u = fpool.tile([128, 40], f32, tag=f"u{half}", name="u")
                nc.vector.reduce_max(
                    u[:, :nb],
                    ps[:, boff : boff + nb * S].rearrange("p (c s) -> p c s", s=S),
                    axis=AX.X,
                )
                nc.vector.tensor_scalar(
                    m[:, slot0 : slot0 + nb], u[:, :nb],
                    tneg[:, t : t + 1], 0.0,
                    op0=OP.add, op1=OP.max,
                )

        def fold_bucket(t, half, r_ap_of, m, buckets):
            """DVE fold tree + reduce for (S, nb, off, slot0) buckets."""
            scr = [
                fpool.tile(
                    [128, 2560 // (2 ** l)], bf16,
                    tag=f"f{half}{l}", name=f"fold{l}",
                )
                for l in range(FOLD_LEVELS)
            ]
            scr_off = [0] * FOLD_LEVELS
            for S, nb, boff, slot0 in buckets:
                widths, Sf = _fold_schedule(S)
                src_ap = r_ap_of(boff, nb * S).rearrange("p (c s) -> p c s", s=S)
                cur_w = S
                for li, wdt in enumerate(widths):
                    o = scr_off[li]
                    dst = scr[li][:, o : o + nb * wdt].rearrange(
                        "p (c s) -> p c s", s=wdt
                    )
                    for _dup in range(DUP_DVE):
                        nc.vector.tensor_max(
                            dst, src_ap[:, :, 0:wdt], src_ap[:, :, wdt:cur_w]
                        )
                    scr_off[li] = o + nb * wdt
                    src_ap = dst
                    cur_w = wdt
                for _dup in range(DUP_DVE):
                    nc.vector.reduce_max(m[:, slot0 : slot0 + nb], src_ap, axis=AX.X)

        def post_matmul(t, half, ps, gw, buckets, m):
            """ACT evict + DVE folds for one tile's generation."""
            r = rpool.tile([128, GEN_CAP], bf16, tag=f"r{half}", name="r")
            for _dup in range(DUP_ACT):
                nc.scalar.activation(
                    r[:, :gw], ps[:, :gw], AF.Relu,
                    bias=tneg[:, t : t + 1], scale=1.0,
                )
            if mode == "mmevict":
                return
            fold_bucket(t, half, lambda off, sz: r[:, off : off + sz], m, buckets)

        def emit_out(t, m):
            # scores = m*inv_norm + th*(m>0):
            #   u = min(m*1e30, th) = th where m>0 else 0;  sc = m*inv_norm + u
            u = opool.tile([128, nslot], f32, tag="g01", name="u")
            nc.vector.tensor_scalar(
                u[:], m[:], 1e30, th, op0=OP.mult, op1=OP.min
            )
            sc = opool.tile([128, nslot], f32, tag="sc", name="sc")
            nc.vector.scalar_tensor_tensor(
                sc[:], m[:], inv[:, t : t + 1], u[:], op0=OP.mult, op1=OP.add
            )
            nc.sync.dma_start(out_d[t * 128 : (t + 1) * 128, :], sc[:])

        # --- main loop: pairs of 128-sample tiles, row-packed on the PE ---
        # Tile A streams through PE rows 0-63 (tile_position (0,0)), tile B
        # through rows 64-127 ((64,0)); with K=64 the interleaved matmuls
        # occupy disjoint row groups and execute concurrently.
        for tp in range(0, T_TILES, 2):
            tiles = [tp, tp + 1] if tp + 1 < T_TILES else [tp]
            ms = {}
            rbigs = {}
            for t in tiles:
                ms[t] = mpool.tile([128, nslot], bf16, tag=f"m{t % 2}", name="m")
                if BIG_R and mode != "mm":
                    rbigs[t] = rpool.tile(
                        [128, Pp], bf16, tag=f"R{t % 2}", name="rbig"
                    )
            col = 0
            for gi, (gw, buckets) in enumerate(gens):
                pss = {}
                for t in tiles:
                    half = t % 2
                    pss[t] = pspool.tile(
                        [128, GEN_CAP], f32, tag=f"ps{half}", name="ps"
                    )
                for j0 in range(0, gw, MM_N):
                    w = min(MM_N, gw - j0)
                    for t in tiles:
                        half = t % 2
                        rs = slice(half * D, (half + 1) * D)
                        cs = slice(t * 128, (t + 1) * 128)
                        js = slice(col + j0, col + j0 + w)
                        dst = pss[t][:, j0 : j0 + w]
                        tpos = (half * D, 0)
                        if MM_DTYPE == "fp32":
                            for _dup in range(DUP_PE):
                                nc.tensor.matmul(
                                    dst, xT[rs, cs], pT[rs, js],
                                    start=(_dup == 0), stop=(_dup == DUP_PE - 1),
                                    tile_position=tpos,
                                )
                        else:
                            # hi*hi + hi*lo + lo*hi accumulated in fp32 PSUM
                            nc.tensor.matmul(
                                dst, xTh[rs, cs], pTh[rs, js],
                                start=True, stop=False, tile_position=tpos,
                            )
                            nc.tensor.matmul(
                                dst, xTh[rs, cs], pTl[rs, js],
                                start=False, stop=False, tile_position=tpos,
                            )
                            nc.tensor.matmul(
                                dst, xTl[rs, cs], pTh[rs, js],
                                start=False, stop=True, tile_position=tpos,
                            )
                if mode != "mm":
                    for t in tiles:
                        if BIG_R:
                            dst = rbigs[t][:, col : col + gw]
                            for _dup in range(DUP_ACT):
                                nc.scalar.activation(
                                    dst, pss[t][:, :gw], AF.Relu,
                                    bias=tneg[:, t : t + 1], scale=1.0,
                                )
                        elif gi in DVE_GENS:
                            post_matmul_dve(t, t % 2, pss[t], gw, buckets, ms[t])
                        else:
                            post_matmul(t, t % 2, pss[t], gw, buckets, ms[t])
                col += gw
            if mode == "mm":
                continue
            if BIG_R:
                for t in tiles:
                    fold_bucket(
                        t, t % 2,
                        lambda off, sz, _t=t: rbigs[_t][:, off : off + sz],
                        ms[t], gbuckets,
                    )
            if mode == "full":
                for t in tiles:
                    emit_out(t, ms[t])
        if rep_ctx is not None:
            rep_ctx.__exit__(None, None, None)

    nc.compile()
    return nc


def _marshal(X, prototypes, order):
    """Build per-core inputs + shared padded transposed prototypes."""
    Pp = order.shape[0]
    pTp = np.zeros((D, Pp), np.float32)
    real = order >= 0
    pTp[:, real] = np.ascontiguousarray(prototypes[order[real]].T)
    pTd = np.concatenate([pTp, pTp], 0)        # duplicated into both halves

    def split16(a):
        hi = a.astype(np.float16)
        lo = (a - hi.astype(np.float32)).astype(np.float16)
        return hi, lo

    pTh, pTl = split16(pTd)
    in_maps = []
    for c in range(N_CORES):
        Xs = np.ascontiguousarray(X[c * N_SH : (c + 1) * N_SH])
        xT = np.ascontiguousarray(Xs.T)
        xTd = np.concatenate([xT, xT], 0)
        m = {"xN": Xs}
        if MM_DTYPE == "fp32":
            m["xT"] = xTd
            m["pT"] = pTd
        else:
            m["xTh"], m["xTl"] = split16(xTd)
            m["pTh"], m["pTl"] = split16(pTd)
        in_maps.append(m)
    return in_maps


def kernel(X, prototypes, sim_th, proto_labels):
    global LAST_EXEC_NS, LAST_MEAN_EXEC_NS
    X = np.asarray(X, np.float32)
    prototypes = np.asarray(prototypes, np.float32)
    sim_th = np.asarray(sim_th, np.float32)
    labels = np.asarray(proto_labels)

    assert X.shape == (N_FULL, D) and prototypes.shape == (P_TOT, D)
    th_vals = np.unique(sim_th)
    assert th_vals.size == 1, "kernel assumes a single global threshold"
    th = float(th_vals[0])

    gens, order, cls_slots, Pp, gbuckets = _plan_layout(labels)
    nslot = len(cls_slots)
    nc = _build_nc(gens, Pp, nslot, th, gbuckets=gbuckets)
    in_maps = _marshal(X, prototypes, order)

    res = run_bass_kernel_spmd(nc, in_maps, list(range(N_CORES)))
    LAST_EXEC_NS = res.exec_time_ns
    LAST_MEAN_EXEC_NS = res.mean_exec_time_ns

    out_sorted = np.concatenate([res.results[i]["out"] for i in range(N_CORES)], 0)
    full = np.zeros((N_FULL, C), np.float32)
    full[:, np.asarray(cls_slots, np.int64)] = out_sorted
    return full
